# revision 31
# baseline (speedup 1.0000x reference)
"""GNN message-passing (GraphConv x4 + mean readout + linear classifier) on 8 TRN2 cores.

Sharding: dst-node (and incident-edge) partitioning across 8 cores. Each layer:
  - every core holds the full node-feature table (fp16) in DRAM, replicated via
    TWO chunked AllGathers (chunk A = each core's local blocks 0..23, chunk B =
    blocks 24..48) so the next layer's chunk-A gathers can start while chunk B
    is still computing/gathering (overlaps the collective with compute).
  - per 128-dst block: dma_gather src rows (fp16), build per-tile selection
    matrices S^T[e,d] = (dst_local[e]==d) on DVE, aggregate m^T via TensorE into
    PSUM, then h' = relu(m @ W + b) and write the core's slice.
Readout: per-block matmul against graph-selection weights (1/cnt folded in),
AllReduce, then classifier matmul.  Dominant traffic: 256B/edge/layer gather.
"""

import math
from contextlib import ExitStack
from dataclasses import dataclass, field

import numpy as np

P = 128  # partitions; also feature dim and max graph count here


# --------------------------------------------------------------------------
# Planning: pack edges into per-core, per-superblock, per-half tile slots.
#
# half of an edge = which chunk of the (relabeled) table its SOURCE lives in:
#   chunk A = rows [0, RA) of every core's slice, chunk B = rows [RA, NPC).
# Flat tile order per core:
#   for each superblock sb: [half0 tiles: blocks in sb, K0_b tiles each]
#                           [half1 tiles: blocks in sb, K1_b tiles each]
# Each tile is 128 slots (one gathered edge row per partition).
# --------------------------------------------------------------------------

@dataclass
class Plan:
    N: int
    E: int
    D: int
    C: int
    G: int
    NC: int
    NPC: int
    NB: int
    CB: int          # chunk boundary (blocks) -> RA = CB*P rows
    SB: int
    n_layers: int
    K0: list = field(default_factory=list)      # per-block half0 tile counts
    K1: list = field(default_factory=list)
    tstart: dict = field(default_factory=dict)  # (b, half) -> first flat tile
    grange: dict = field(default_factory=dict)  # (sb, half) -> (tile_lo, ntiles)
    gtrim: dict = field(default_factory=dict)   # (sb, half) -> live slot count
    ntiles: int = 0
    src16: list = field(default_factory=list)   # [P, slots//16] int16 (row-replicated x8)
    dl: list = field(default_factory=list)      # [P, ntiles] fp16 dst_local (128=pad)
    scc: list = field(default_factory=list)     # [P, NB] fp32 c_src*c_dst per node
    scd: list = field(default_factory=list)     # [P, NB] fp32 c_dst per node
    icd: list = field(default_factory=list)     # [1, NB*P] fp16 1/c_dst per node
    gidf: list = field(default_factory=list)    # [P, NB] fp32 graph id per node
    invc: list = field(default_factory=list)    # [P, NB] fp32 1/cnt per node

    @property
    def RA(self):
        return self.CB * P

    @property
    def RB(self):
        return self.NPC - self.RA

    @property
    def nsb(self):
        return math.ceil(self.NB / self.SB)

    def sb_blocks(self, sb):
        return range(sb * self.SB, min((sb + 1) * self.SB, self.NB))

    @property
    def slots(self):
        return self.ntiles * P

    @property
    def max_sb_tiles(self):
        return max(self.grange[(sb, 0)][1] + self.grange[(sb, 1)][1]
                   for sb in range(self.nsb))


def make_plan(x, edge_index, graph_ids, n_layers=4, NC=8, SB=2, CB=31, C=10, G=None,
              negpad=True):
    N, D = x.shape
    E = edge_index.shape[1]
    if G is None:
        G = int(np.asarray(graph_ids).max()) + 1
    assert G <= P and D == P
    src = np.asarray(edge_index[0], dtype=np.int64)
    dst = np.asarray(edge_index[1], dtype=np.int64)
    NPC = math.ceil(N / NC)
    NB = math.ceil(NPC / P)
    assert 0 < CB < NB
    RA, RB = CB * P, NPC - CB * P

    out_deg = np.bincount(src, minlength=N).astype(np.float64)
    in_deg = np.bincount(dst, minlength=N).astype(np.float64)
    c_src = np.clip(out_deg, 1.0, None) ** -0.5
    c_dst = np.clip(in_deg, 1.0, None) ** -0.5

    core_of_edge = dst // NPC
    blk_in_core = (dst - core_of_edge * NPC) // P
    src_core = src // NPC
    src_r = src - src_core * NPC
    half_of_edge = (src_r >= RA).astype(np.int64)
    # table index within its chunk (relabeled by core-major chunk layout)
    tab_idx = np.where(half_of_edge == 0,
                       src_core * RA + src_r,
                       src_core * RB + (src_r - RA))
    assert tab_idx.max() < 32767

    key = np.lexsort((src, half_of_edge, blk_in_core, core_of_edge))
    tab_s = tab_idx[key]
    dst_s = dst[key]
    half_s = half_of_edge[key]
    core_s = core_of_edge[key]
    blk_s = blk_in_core[key]

    plan = Plan(N=N, E=E, D=D, C=C, G=G, NC=NC, NPC=NPC, NB=NB, CB=CB,
                SB=SB, n_layers=n_layers)

    # per-(core, block, half) edge masks and tile counts (max over cores so the
    # flat tile layout is identical on every core -> one compiled kernel)
    per_block = {}
    K0 = np.ones(NB, dtype=np.int64)
    K1 = np.ones(NB, dtype=np.int64)
    for c in range(NC):
        mc = core_s == c
        for b in range(NB):
            mb = mc & (blk_s == b)
            e0 = int((half_s[mb] == 0).sum())
            e1 = int(mb.sum()) - e0
            per_block[(c, b)] = mb
            K0[b] = max(K0[b], math.ceil(max(e0, 1) / P))
            K1[b] = max(K1[b], math.ceil(max(e1, 1) / P))
    plan.K0, plan.K1 = K0.tolist(), K1.tolist()

    # flat tile layout
    nt = 0
    for sb in range(plan.nsb):
        blocks = list(plan.sb_blocks(sb))
        lo0 = nt
        for b in blocks:
            plan.tstart[(b, 0)] = nt
            nt += K0[b]
        plan.grange[(sb, 0)] = (lo0, nt - lo0)
        lo1 = nt
        for b in blocks:
            plan.tstart[(b, 1)] = nt
            nt += K1[b]
        plan.grange[(sb, 1)] = (lo1, nt - lo1)
    plan.ntiles = nt

    cnt = np.bincount(np.asarray(graph_ids, dtype=np.int64), minlength=G).astype(np.float64)
    invc_all = 1.0 / np.clip(cnt, 1.0, None)

    plan.c_src, plan.c_dst = c_src.astype(np.float32), c_dst.astype(np.float32)
    ntiles = plan.ntiles
    for c in range(NC):
        # pad slots use idx -1: the gather ucode trims trailing negatives, so
        # pad at the tail of each (sb, half) range is neither generated nor
        # drained. Interior pad (non-final blocks) still gathers row 0.
        src_flat = np.full(ntiles * P, -1 if negpad else 0, dtype=np.int16)
        dl_flat = np.full(ntiles * P, 128.0, dtype=np.float16)  # 128 = no match
        for b in range(NB):
            mb = per_block[(c, b)]
            t_b, d_b, h_b = tab_s[mb], dst_s[mb], half_s[mb]
            dloc = (d_b - c * NPC - b * P).astype(np.float16)
            for half in (0, 1):
                sel = h_b == half
                idx = t_b[sel]
                base = plan.tstart[(b, half)] * P
                n = len(idx)
                src_flat[base:base + n] = idx.astype(np.int16)
                dl_flat[base:base + n] = dloc[sel]
        if negpad:
            for sb in range(plan.nsb):
                for half in (0, 1):
                    lo, ntl = plan.grange[(sb, half)]
                    seg = src_flat[lo * P:(lo + ntl) * P]
                    last = np.nonzero(seg >= 0)[0]
                    keep = (int(last[-1]) + 1) if len(last) else 0
                    seg[:keep][seg[:keep] < 0] = 0  # interior pad -> row 0
                    plan.gtrim[(sb, half)] = max(plan.gtrim.get((sb, half), 1),
                                                 keep)
        else:
            for sb in range(plan.nsb):
                for half in (0, 1):
                    plan.gtrim[(sb, half)] = plan.grange[(sb, half)][1] * P

        wrapped = src_flat.reshape(-1, 16).T          # [16, slots/16]
        plan.src16.append(np.ascontiguousarray(np.tile(wrapped, (8, 1))))
        plan.dl.append(np.ascontiguousarray(dl_flat.reshape(-1, P).T))   # [P, ntiles]
        # per-block-node scale/bias-fold arrays
        lo, hi = c * NPC, min((c + 1) * NPC, N)
        cs = np.zeros(NB * P, np.float32); cs[:hi - lo] = c_src[lo:hi]
        cd = np.ones(NB * P, np.float32); cd[:hi - lo] = c_dst[lo:hi]
        plan.scc.append(np.ascontiguousarray((cs * cd).reshape(NB, P).T))  # [P,NB]
        plan.scd.append(np.ascontiguousarray(cd.reshape(NB, P).T))         # [P,NB]
        plan.icd.append(np.ascontiguousarray((1.0 / cd).reshape(1, NB * P).astype(np.float16)))

        gidf = np.zeros(NB * P, dtype=np.float32)
        invc = np.zeros(NB * P, dtype=np.float32)
        gids = np.asarray(graph_ids[lo:hi], dtype=np.int64)
        gidf[:hi - lo] = gids.astype(np.float32)
        invc[:hi - lo] = invc_all[gids].astype(np.float32)
        plan.gidf.append(np.ascontiguousarray(gidf.reshape(NB, P).T))    # [P, NB]
        plan.invc.append(np.ascontiguousarray(invc.reshape(NB, P).T))    # [P, NB]

    return plan


def split_table(plan: Plan, tab):
    """[N, D] node table -> (chunk A [NC*RA, D], chunk B [NC*RB, D])."""
    NPC, RA = plan.NPC, plan.RA
    t = tab.reshape(plan.NC, NPC, -1)
    a = t[:, :RA, :].reshape(plan.NC * RA, -1)
    b = t[:, RA:, :].reshape(plan.NC * plan.RB, -1)
    return np.ascontiguousarray(a), np.ascontiguousarray(b)


# --------------------------------------------------------------------------
# Golden numpy model of the exact device algorithm (fp16 gather/aggregation).
# --------------------------------------------------------------------------

def golden(plan: Plan, x, W_all, b_all, Wc, bc):
    f16, f32 = np.float16, np.float32
    htabA, htabB = split_table(plan, (x * plan.c_src[:, None]).astype(f16))
    NPC, NB = plan.NPC, plan.NB
    iota = np.arange(P, dtype=f16)[None, :]
    h4_blocks = [[None] * NB for _ in range(plan.NC)]
    for layer in range(plan.n_layers):
        W16 = W_all[layer].astype(f16)
        nxtA = np.zeros_like(htabA)
        nxtB = np.zeros_like(htabB)
        for c in range(plan.NC):
            flat_idx = plan.src16[c][:16, :].T.reshape(-1)
            dl = plan.dl[c]
            for b in range(NB):
                mT = np.zeros((plan.D, P), dtype=f32)
                for half, K in ((0, plan.K0[b]), (1, plan.K1[b])):
                    htab = htabA if half == 0 else htabB
                    for t in range(K):
                        ti = plan.tstart[(b, half)] + t
                        sl = flat_idx[ti * P:(ti + 1) * P].astype(np.int64)
                        M = htab[sl, :]
                        ST = (iota == dl[:, ti:ti + 1]).astype(f16)
                        mT += M.astype(f32).T @ ST.astype(f32)
                mT16 = mT.astype(f16)
                pre = mT16.astype(f32).T @ W16.astype(f32)
                icd = plan.icd[c][0, b * P:(b + 1) * P].astype(f32)
                pre += icd[:, None] @ b_all[layer].astype(f32)[None, :]
                scl = (plan.scc[c] if layer < plan.n_layers - 1 else
                       plan.scd[c])[:, b]
                hb = np.maximum(pre * scl[:, None], 0).astype(f16)
                h4_blocks[c][b] = hb
                lo = b * P
                hi = min(lo + P, NPC)
                rows = hi - lo
                if lo < plan.RA:
                    nxtA[c * plan.RA + lo:c * plan.RA + lo + rows] = hb[:rows]
                else:
                    o = lo - plan.RA
                    nxtB[c * plan.RB + o:c * plan.RB + o + rows] = hb[:rows]
        htabA, htabB = nxtA, nxtB
    pgT = np.zeros((plan.D, P), dtype=f32)
    for c in range(plan.NC):
        for b in range(NB):
            hb = h4_blocks[c][b]
            SgT = ((iota.astype(np.float32) == plan.gidf[c][:, b:b + 1]) * plan.invc[c][:, b:b + 1]).astype(f16)
            pgT += hb.astype(f32).T @ SgT.astype(f32)
    out = pgT.T @ Wc.astype(f32) + bc[None, :]
    return out[:plan.G].astype(f32)


# --------------------------------------------------------------------------
# Bass/Tile kernel builder.
# --------------------------------------------------------------------------

def build_inputs(plan: Plan, x, W_all, b_all, Wc, bc):
    """Per-core in_maps for run_bass_kernel_spmd."""
    iota = np.tile(np.arange(P, dtype=np.float16)[None, :], (P, 1))
    xA, xB = split_table(plan, (x * plan.c_src[:, None]).astype(np.float16))
    pad = np.zeros((1, xA.shape[1]), xA.dtype)
    common = {
        "xA16": np.ascontiguousarray(np.concatenate([xA, pad], 0)),
        "xB16": np.ascontiguousarray(np.concatenate([xB, pad], 0)),
        "w16": np.ascontiguousarray(W_all.astype(np.float16)),
        "b16": np.ascontiguousarray(b_all.astype(np.float16).reshape(1, -1)),
        "wc32": np.ascontiguousarray(Wc.astype(np.float32)),
        "bc32": np.ascontiguousarray(bc.astype(np.float32).reshape(1, -1)),
        "iota16": iota,
    }
    maps = []
    for c in range(plan.NC):
        m = dict(common)
        m["src16"] = plan.src16[c]
        m["dl16"] = plan.dl[c]
        m["scc32"] = plan.scc[c]
        m["scd32"] = plan.scd[c]
        m["icd16"] = plan.icd[c]
        m["gid16"] = plan.gidf[c]
        m["ivc16"] = plan.invc[c]
        maps.append(m)
    return maps


def build_nc(plan: Plan, num_swdge_queues=4, use_collectives=True,
             single_packet=False, scratch_size=49152, g_bufs=5,
             iota_big=False, elem2=False):
    import concourse.bass as bass
    import concourse.tile as tile
    from concourse import bacc, mybir
    from concourse.tile_rust import add_dep_helper

    def _inst(i):
        return i.ins if hasattr(i, "ins") and not hasattr(i, "engine") else i

    f16, f32, i16 = mybir.dt.float16, mybir.dt.float32, mybir.dt.int16
    NB, SB, CB = plan.NB, plan.SB, plan.CB
    NL = plan.n_layers
    KT = max(plan.K0[b] + plan.K1[b] for b in range(NB))
    NTA, NTB = plan.NC * plan.RA, plan.NC * plan.RB

    nc = bacc.Bacc(
        "TRN2",
        target_bir_lowering=False,
        debug=False,
        num_devices=plan.NC,
        num_swdge_queues=num_swdge_queues,
        dynamic_dma_scratch_size=scratch_size,
    )
    rg = [list(range(plan.NC))]

    # ---- DRAM I/O ----
    # tables carry one pad row so elem2 (512B/descriptor) can read row+1
    PAD = 1
    xA16 = nc.dram_tensor("xA16", [NTA + PAD, P], f16, kind="ExternalInput")
    xB16 = nc.dram_tensor("xB16", [NTB + PAD, P], f16, kind="ExternalInput")
    w16 = nc.dram_tensor("w16", [NL, P, P], f16, kind="ExternalInput")
    b16 = nc.dram_tensor("b16", [1, NL * P], f16, kind="ExternalInput")
    wc32 = nc.dram_tensor("wc32", [P, plan.C], f32, kind="ExternalInput")
    bc32 = nc.dram_tensor("bc32", [1, plan.C], f32, kind="ExternalInput")
    iota16 = nc.dram_tensor("iota16", [P, P], f16, kind="ExternalInput")
    src16 = nc.dram_tensor("src16", [P, plan.slots // 16], i16, kind="ExternalInput")
    dl16 = nc.dram_tensor("dl16", [P, plan.ntiles], f16, kind="ExternalInput")
    scc32 = nc.dram_tensor("scc32", [P, NB], f32, kind="ExternalInput")
    scd32 = nc.dram_tensor("scd32", [P, NB], f32, kind="ExternalInput")
    icd16 = nc.dram_tensor("icd16", [1, NB * P], f16, kind="ExternalInput")
    gid16 = nc.dram_tensor("gid16", [P, NB], f32, kind="ExternalInput")
    ivc16 = nc.dram_tensor("ivc16", [P, NB], f32, kind="ExternalInput")
    out_d = nc.dram_tensor("out", [plan.G, plan.C], f32, kind="ExternalOutput")

    # internal DRAM: per-layer local chunk slices + gathered chunk tables
    hlocA = [nc.dram_tensor(f"hlocA{l}", [plan.RA, P], f16) for l in range(NL - 1)]
    hlocB = [nc.dram_tensor(f"hlocB{l}", [plan.RB, P], f16) for l in range(NL - 1)]
    tabA = [nc.dram_tensor(f"tabA{l}", [NTA + PAD, P], f16, addr_space="Shared")
            for l in range(NL - 1)]
    tabB = [nc.dram_tensor(f"tabB{l}", [NTB + PAD, P], f16, addr_space="Shared")
            for l in range(NL - 1)]
    pg_in = nc.dram_tensor("pg_in", [P, P], f32)
    pg_out = nc.dram_tensor("pg_out", [P, P], f32, addr_space="Shared")

    with tile.TileContext(nc) as tc, ExitStack() as ctx:
        const = ctx.enter_context(tc.tile_pool(name="const", bufs=1))
        gpool = ctx.enter_context(tc.tile_pool(name="gather", bufs=g_bufs))
        spool = ctx.enter_context(tc.tile_pool(name="sel", bufs=8))
        mpool = ctx.enter_context(tc.tile_pool(name="mt", bufs=4))
        hpool = ctx.enter_context(tc.tile_pool(name="hb", bufs=6))
        psum_m = ctx.enter_context(tc.tile_pool(name="psum_m", bufs=3, space="PSUM"))
        psum_h = ctx.enter_context(tc.tile_pool(name="psum_h", bufs=2, space="PSUM"))
        psum_g = ctx.enter_context(tc.tile_pool(name="psum_g", bufs=1, space="PSUM"))
        opool = ctx.enter_context(tc.tile_pool(name="outp", bufs=1))

        # ---- constants into SBUF ----
        def cload(tag, dram, shape, dt):
            t = const.tile(shape, dt, tag=tag)
            nc.sync.dma_start(out=t[:], in_=dram[:])
            return t

        IOTA = cload("iota", iota16, [P, P], f16)
        SRC = cload("src", src16, [P, plan.slots // 16], i16)
        DL = cload("dl", dl16, [P, plan.ntiles], f16)
        SCC = cload("scc", scc32, [P, NB], f32)
        SCD = cload("scd", scd32, [P, NB], f32)
        ICD = cload("icd", icd16, [1, NB * P], f16)
        GID = cload("gid", gid16, [P, NB], f32)
        IVC = cload("ivc", ivc16, [P, NB], f32)
        WTS = [cload(f"wt{l}", w16[l], [P, P], f16) for l in range(NL)]
        BROW = cload("brow", b16, [1, NL * P], f16)
        WC = cload("wc", wc32, [P, plan.C], f32)
        BC = cload("bc", bc32, [1, plan.C], f32)
        ONE16 = const.tile([1, P], f16, tag="one16")
        nc.vector.memset(ONE16[:], 1.0)
        ONE32 = const.tile([1, P], f32, tag="one32")
        nc.vector.memset(ONE32[:], 1.0)

        is_eq = mybir.AluOpType.is_equal
        mult = mybir.AluOpType.mult

        KMAX = max(max(plan.K0), max(plan.K1))
        IOTAB = None
        if iota_big:
            IOTAB = const.tile([P, KMAX, P], f16, tag="iotab")
            nc.vector.tensor_copy(
                out=IOTAB[:],
                in_=IOTA[:].unsqueeze(1).to_broadcast([P, KMAX, P]))

        def sel_tile(col_src, col):
            """Sg^T[n,g] = (iota_g == gid[n]) * invc[n], fp16 (readout only)."""
            st = spool.tile([P, P], f16, tag="sel")
            nc.vector.tensor_scalar(
                out=st[:], in0=IOTA[:],
                scalar1=col_src[0][:, col:col + 1],
                scalar2=col_src[1][:, col:col + 1],
                op0=is_eq, op1=mult)
            return st

        def sel_batch(t0, nt):
            """Binary S^T for nt consecutive tiles: [P, nt, P] fp16."""
            st = spool.tile([P, KMAX, P], f16, tag="selb")
            in0 = (IOTAB[:, :nt, :] if IOTAB is not None else
                   IOTA[:].unsqueeze(1).to_broadcast([P, nt, P]))
            nc.vector.tensor_tensor(
                out=st[:, :nt, :],
                in0=in0,
                in1=DL[:, t0:t0 + nt].unsqueeze(2).to_broadcast([P, nt, P]),
                op=is_eq)
            return st

        # collective instructions whose outputs feed gathers: (layer-1) -> inst
        cc_a = [None] * NL
        cc_b = [None] * NL
        pg = psum_g.tile([P, P], f32, tag="pg")  # readout accumulator
        gq = 0  # round-robin queue counter
        PF = min(5, g_bufs - 2)  # superblocks prefetched (half0) before cc_b

        GW = 256 if elem2 else 128  # gathered row width (elems) per slot

        # zero the gather buffers once: slots trimmed by negative-index padding
        # stay unwritten, and zero × zero-sel avoids NaN from stale SBUF
        for _ in range(g_bufs):
            gz = gpool.tile([P, plan.max_sb_tiles, GW], f16, tag="g")
            nc.vector.memset(gz[:], 0.0)

        def _overlap_ap(tab):
            # [R+1, 128] -> overlapping [R, 256] view with row stride 128
            import bass_rust
            t2 = tab.copy()
            rows = t2.ap[0][1] - 1
            t2.ap = bass_rust.VecI64Pair([(P, rows), (1, 2 * P)])
            return t2

        def emit_gather(g, sb, half, tab, dep):
            nonlocal gq
            gt0 = plan.grange[(sb, 0)][0]
            tile_lo, ntile_h = plan.grange[(sb, half)]
            n_idx = ntile_h * P
            col0 = tile_lo * P // 16
            toff = tile_lo - gt0
            gi = nc.gpsimd.dma_gather(
                g[:, toff:toff + ntile_h, :],
                _overlap_ap(tab) if elem2 else tab,
                SRC[:, col0:col0 + n_idx // 16],
                n_idx, plan.gtrim[(sb, half)], GW,
                elem_step=P if elem2 else None,
                queue_num=gq % num_swdge_queues,
                single_packet=single_packet,
            )
            gq += 1
            if dep is not None:
                add_dep_helper(_inst(gi), _inst(dep), reason="gather after AG")
            return gi

        for layer in range(NL):
            tA = (xA16 if layer == 0 else tabA[layer - 1])[0:NTA + PAD, :]
            tB = (xB16 if layer == 0 else tabB[layer - 1])[0:NTB + PAD, :]
            with nc.named_scope(f"conv{layer}"):
                # prefetch: half0 gathers for the first PF superblocks (they
                # only need chunk A, whose AllGather fired mid-previous-layer),
                # then trigger chunk B's AllGather (stalls on the previous
                # layer's tail compute while the prefetched gathers drain).
                gtiles = {}
                for sb in range(min(PF, plan.nsb)):
                    g = gpool.tile([P, plan.max_sb_tiles, GW], f16, tag="g")
                    gtiles[sb] = g
                    emit_gather(g, sb, 0, tA, cc_a[layer])
                if layer >= 1 and use_collectives:
                    cc_b[layer] = nc.gpsimd.collective_compute(
                        "AllGather", mybir.AluOpType.bypass,
                        ins=[hlocB[layer - 1].ap().opt()],
                        outs=[tabB[layer - 1][0:NTB, :].opt()],
                        replica_groups=rg)

                for sb in range(plan.nsb):
                    blocks = list(plan.sb_blocks(sb))
                    gt0 = plan.grange[(sb, 0)][0]  # flat tile base of this sb
                    if sb in gtiles:
                        g = gtiles.pop(sb)
                    else:
                        g = gpool.tile([P, plan.max_sb_tiles, GW], f16, tag="g")
                        emit_gather(g, sb, 0, tA, cc_a[layer])
                    emit_gather(g, sb, 1, tB, cc_b[layer])

                    for b in blocks:
                        pm = psum_m.tile([P, P], f32, tag="pm")
                        k0, k1 = plan.K0[b], plan.K1[b]
                        st0 = sel_batch(plan.tstart[(b, 0)], k0)
                        st1 = sel_batch(plan.tstart[(b, 1)], k1)
                        for t in range(k0 + k1):
                            st = st0[:, t, :] if t < k0 else st1[:, t - k0, :]
                            gt = (plan.tstart[(b, 0)] + t if t < k0 else
                                  plan.tstart[(b, 1)] + (t - k0)) - gt0
                            nc.tensor.matmul(
                                out=pm[:], lhsT=g[:, gt, 0:P], rhs=st,
                                start=(t == 0), stop=(t == k0 + k1 - 1))
                        mt = mpool.tile([P, P], f16, tag="mt")
                        nc.vector.tensor_copy(out=mt[:], in_=pm[:])
                        ph = psum_h.tile([P, P], f32, tag="ph")
                        nc.tensor.matmul(out=ph[:], lhsT=mt[:], rhs=WTS[layer][:],
                                         start=True, stop=False)
                        nc.tensor.matmul(out=ph[:],
                                         lhsT=ICD[0:1, b * P:(b + 1) * P],
                                         rhs=BROW[0:1, layer * P:(layer + 1) * P],
                                         start=False, stop=True)
                        hb = hpool.tile([P, P], f16, tag="hb")
                        scl = SCC if layer < NL - 1 else SCD
                        nc.scalar.activation(
                            out=hb[:], in_=ph[:],
                            func=mybir.ActivationFunctionType.Relu,
                            scale=scl[:, b:b + 1])
                        if layer < NL - 1:
                            if b < CB:
                                rows = P
                                nc.sync.dma_start(
                                    out=hlocA[layer][b * P:b * P + rows, :],
                                    in_=hb[:rows, :])
                            else:
                                o = (b - CB) * P
                                rows = min(plan.RB - o, P)
                                nc.sync.dma_start(
                                    out=hlocB[layer][o:o + rows, :],
                                    in_=hb[:rows, :])
                        else:
                            # fold the mean-readout accumulation in here so
                            # only the AllReduce + classifier remain at the end
                            sg = sel_tile((GID, IVC), b)
                            nc.tensor.matmul(out=pg[:], lhsT=hb[:], rhs=sg[:],
                                             start=(b == 0), stop=(b == NB - 1))

                    # fire chunk A's AllGather as soon as blocks 0..CB-1 are
                    # written (chunk B's AllGather is emitted at the start of
                    # the next layer, after its prefetch gathers)
                    if (layer < NL - 1 and use_collectives
                            and blocks[0] <= CB - 1 <= blocks[-1]):
                        cc_a[layer + 1] = nc.gpsimd.collective_compute(
                            "AllGather", mybir.AluOpType.bypass,
                            ins=[hlocA[layer].ap().opt()],
                            outs=[tabA[layer][0:NTA, :].opt()],
                            replica_groups=rg)

        # ---- readout ----
        with nc.named_scope("readout"):
            pgs = opool.tile([P, P], f32, tag="pgs")
            nc.vector.tensor_copy(out=pgs[:], in_=pg[:])
            wr = nc.sync.dma_start(out=pg_in[:, :], in_=pgs[:])
            if use_collectives:
                cc = nc.gpsimd.collective_compute(
                    "AllReduce", mybir.AluOpType.add,
                    ins=[pg_in.ap().opt()], outs=[pg_out.ap().opt()],
                    replica_groups=rg)
            else:
                cc = nc.sync.dma_start(out=pg_out[:, :], in_=pg_in[:, :])
            hgT = opool.tile([P, P], f32, tag="hgT")
            rd = nc.sync.dma_start(out=hgT[:], in_=pg_out[:, :])
            add_dep_helper(_inst(rd), _inst(cc), reason="read after AR")
            po = psum_g.tile([P, plan.C], f32, tag="po")
            nc.tensor.matmul(out=po[:plan.G, :], lhsT=hgT[:, :plan.G], rhs=WC[:],
                             start=True, stop=False)
            nc.tensor.matmul(out=po[:plan.G, :], lhsT=ONE32[0:1, :plan.G], rhs=BC[:],
                             start=False, stop=True)
            ob = opool.tile([P, plan.C], f32, tag="ob")
            nc.vector.tensor_copy(out=ob[:plan.G, :], in_=po[:plan.G, :])
            nc.sync.dma_start(out=out_d[:, :], in_=ob[:plan.G, :])

    nc.compile()
    return nc


# --------------------------------------------------------------------------
# Entry point.
# --------------------------------------------------------------------------

_CACHE = {}


def _get_compiled(plan_key, plan):
    if plan_key not in _CACHE:
        _CACHE[plan_key] = build_nc(plan, g_bufs=8)
    return _CACHE[plan_key]


def kernel(x, W0, b0, Ws, bs, Wc, bc, edge_index, graph_ids):
    x = np.asarray(x)
    edge_index = np.asarray(edge_index)
    graph_ids = np.asarray(graph_ids)
    W_all = np.concatenate([np.asarray(W0)[None], np.asarray(Ws)], axis=0)
    b_all = np.concatenate([np.asarray(b0)[None], np.asarray(bs)], axis=0)
    Wc, bc = np.asarray(Wc), np.asarray(bc)

    plan = make_plan(x, edge_index, graph_ids, SB=2)
    key = (plan.N, plan.E, plan.G, plan.ntiles, plan.n_layers)
    nc = _get_compiled(key, plan)

    from concourse.bass_utils import run_bass_kernel_spmd
    in_maps = build_inputs(plan, x, W_all, b_all, Wc, bc)
    res = run_bass_kernel_spmd(nc, in_maps, core_ids=list(range(plan.NC)))
    return res.results[0]["out"].astype(np.float32)


# revision 34
# speedup vs baseline: 1.3462x; 1.3462x over previous
"""GNN message-passing (GraphConv x4 + mean readout + linear classifier) on 8 TRN2 cores.

Sharding: dst-node (and incident-edge) partitioning across 8 cores. Each layer:
  - every core holds the full node-feature table (fp16) in DRAM (replicated via AllGather)
  - per 128-dst block: dma_gather src rows (fp16), build per-tile selection matrices
    S^T[e,d] = w_e * (dst_local[e]==d) on DVE, aggregate m^T via TensorE into PSUM,
    then h' = relu(m @ W + b) and write the core's slice; AllGather -> next table.
Readout: per-block matmul against graph-selection weights (1/cnt folded in),
AllReduce, then classifier matmul.  Dominant traffic: 256B/edge/layer gather.
"""

import math
from contextlib import ExitStack
from dataclasses import dataclass, field

import numpy as np

P = 128  # partitions; also feature dim and max graph count here


# --------------------------------------------------------------------------
# Planning: pack edges into per-core, per-superblock, per-half tile slots.
#
# Flat slot order per core:
#   for each superblock sb (SB blocks):
#     [all half0 tiles: block b0..b_last, K0 tiles each]
#     [all half1 tiles: block b0..b_last, K1 tiles each]
# Each tile is 128 slots (one gathered edge row per partition).
# --------------------------------------------------------------------------

@dataclass
class Plan:
    N: int
    E: int
    D: int
    C: int
    G: int
    NC: int
    NPC: int
    NB: int
    K0: int
    K1: int
    SB: int
    HALF: int
    n_layers: int
    src16: list = field(default_factory=list)   # [P, slots//16] int16 (row-replicated x8)
    dl: list = field(default_factory=list)      # [P, ntiles] fp16 dst_local (128=pad)
    scc: list = field(default_factory=list)     # [P, NB] fp32 c_src*c_dst per node
    scd: list = field(default_factory=list)     # [P, NB] fp32 c_dst per node
    icd: list = field(default_factory=list)     # [1, NB*P] fp16 1/c_dst per node
    gidf: list = field(default_factory=list)    # [P, NB] fp32 graph id per node
    invc: list = field(default_factory=list)    # [P, NB] fp32 1/cnt per node

    @property
    def nsb(self):
        return math.ceil(self.NB / self.SB)

    def sb_blocks(self, sb):
        return range(sb * self.SB, min((sb + 1) * self.SB, self.NB))

    @property
    def ntiles(self):
        return self.NB * (self.K0 + self.K1)

    @property
    def slots(self):
        return self.ntiles * P

    def tile_index(self, b, t):
        """Global tile index for block b, per-block tile t (t<K0: half0)."""
        sb, bl = b // self.SB, b % self.SB
        nblk = len(self.sb_blocks(sb))
        base = sb * self.SB * (self.K0 + self.K1)  # tiles before this sb
        if t < self.K0:
            return base + bl * self.K0 + t
        return base + nblk * self.K0 + bl * self.K1 + (t - self.K0)


def make_plan(x, edge_index, graph_ids, n_layers=4, NC=8, SB=2, C=10, G=None):
    N, D = x.shape
    E = edge_index.shape[1]
    if G is None:
        G = int(np.asarray(graph_ids).max()) + 1
    assert G <= P and D == P
    src = np.asarray(edge_index[0], dtype=np.int64)
    dst = np.asarray(edge_index[1], dtype=np.int64)
    NPC = math.ceil(N / NC)
    NB = math.ceil(NPC / P)
    HALF = math.ceil(N / 2)
    assert HALF <= 32767 and N - HALF <= 32767

    out_deg = np.bincount(src, minlength=N).astype(np.float64)
    in_deg = np.bincount(dst, minlength=N).astype(np.float64)
    c_src = np.clip(out_deg, 1.0, None) ** -0.5
    c_dst = np.clip(in_deg, 1.0, None) ** -0.5
    w_all = (c_src[src] * c_dst[dst]).astype(np.float32)

    core_of_edge = dst // NPC
    blk_in_core = (dst - core_of_edge * NPC) // P
    half_of_edge = (src >= HALF).astype(np.int64)
    key = np.lexsort((src, half_of_edge, blk_in_core, core_of_edge))
    src_s, w_s = src[key], w_all[key]
    dst_s = dst[key]
    half_s = half_of_edge[key]
    core_s = core_of_edge[key]
    blk_s = blk_in_core[key]

    K0 = K1 = 1
    per_block = {}
    for c in range(NC):
        mc = core_s == c
        for b in range(NB):
            mb = mc & (blk_s == b)
            e0 = int((half_s[mb] == 0).sum())
            e1 = int(mb.sum()) - e0
            per_block[(c, b)] = mb
            K0 = max(K0, math.ceil(max(e0, 1) / P))
            K1 = max(K1, math.ceil(max(e1, 1) / P))

    plan = Plan(N=N, E=E, D=D, C=C, G=G, NC=NC, NPC=NPC, NB=NB, K0=K0, K1=K1,
                SB=SB, HALF=HALF, n_layers=n_layers)

    cnt = np.bincount(np.asarray(graph_ids, dtype=np.int64), minlength=G).astype(np.float64)
    invc_all = 1.0 / np.clip(cnt, 1.0, None)

    # per-node normalization folded into tables/activations:
    #   table_l = c_src * h_l ; h' = relu(cc * (m_raw @ W + invcd x b))
    plan.c_src, plan.c_dst = c_src.astype(np.float32), c_dst.astype(np.float32)
    ntiles = plan.ntiles
    for c in range(NC):
        src_flat = np.zeros(ntiles * P, dtype=np.int16)
        dl_flat = np.full(ntiles * P, 128.0, dtype=np.float16)  # 128 = no match
        for b in range(NB):
            mb = per_block[(c, b)]
            s_b, d_b, h_b = src_s[mb], dst_s[mb], half_s[mb]
            dloc = (d_b - c * NPC - b * P).astype(np.float16)
            for half, K in ((0, K0), (1, K1)):
                sel = h_b == half
                idx = s_b[sel] - (HALF if half else 0)
                t0 = plan.tile_index(b, 0 if half == 0 else K0)
                base = t0 * P
                n = len(idx)
                src_flat[base:base + n] = idx.astype(np.int16)
                dl_flat[base:base + n] = dloc[sel]

        wrapped = src_flat.reshape(-1, 16).T          # [16, slots/16]
        plan.src16.append(np.ascontiguousarray(np.tile(wrapped, (8, 1))))
        plan.dl.append(np.ascontiguousarray(dl_flat.reshape(-1, P).T))   # [P, ntiles]
        # per-block-node scale/bias-fold arrays
        lo, hi = c * NPC, min((c + 1) * NPC, N)
        cs = np.zeros(NB * P, np.float32); cs[:hi - lo] = c_src[lo:hi]
        cd = np.ones(NB * P, np.float32); cd[:hi - lo] = c_dst[lo:hi]
        plan.scc.append(np.ascontiguousarray((cs * cd).reshape(NB, P).T))  # [P,NB]
        plan.scd.append(np.ascontiguousarray(cd.reshape(NB, P).T))         # [P,NB]
        plan.icd.append(np.ascontiguousarray((1.0 / cd).reshape(1, NB * P).astype(np.float16)))

        gidf = np.zeros(NB * P, dtype=np.float32)
        invc = np.zeros(NB * P, dtype=np.float32)
        lo, hi = c * NPC, min((c + 1) * NPC, N)
        gids = np.asarray(graph_ids[lo:hi], dtype=np.int64)
        gidf[:hi - lo] = gids.astype(np.float32)
        invc[:hi - lo] = invc_all[gids].astype(np.float32)
        plan.gidf.append(np.ascontiguousarray(gidf.reshape(NB, P).T))    # [P, NB]
        plan.invc.append(np.ascontiguousarray(invc.reshape(NB, P).T))    # [P, NB]

    return plan


# --------------------------------------------------------------------------
# Golden numpy model of the exact device algorithm (fp16 gather/aggregation).
# --------------------------------------------------------------------------

def golden(plan: Plan, x, W_all, b_all, Wc, bc):
    f16, f32 = np.float16, np.float32
    htab = (x * plan.c_src[:, None]).astype(f16)
    NPC, NB, K0, K1, HALF = plan.NPC, plan.NB, plan.K0, plan.K1, plan.HALF
    KT = K0 + K1
    iota = np.arange(P, dtype=f16)[None, :]
    h4_blocks = [[None] * NB for _ in range(plan.NC)]
    for layer in range(plan.n_layers):
        W16 = W_all[layer].astype(f16)
        nxt = np.zeros((plan.N, plan.D), dtype=f16)
        for c in range(plan.NC):
            flat_idx = plan.src16[c][:16, :].T.reshape(-1)
            dl = plan.dl[c]
            for b in range(NB):
                mT = np.zeros((plan.D, P), dtype=f32)
                for t in range(KT):
                    ti = plan.tile_index(b, t)
                    sl = flat_idx[ti * P:(ti + 1) * P].astype(np.int64)
                    base = 0 if t < K0 else HALF
                    M = htab[base + sl, :]
                    ST = (iota == dl[:, ti:ti + 1]).astype(f16)
                    mT += M.astype(f32).T @ ST.astype(f32)
                mT16 = mT.astype(f16)
                pre = mT16.astype(f32).T @ W16.astype(f32)
                icd = plan.icd[c][0, b * P:(b + 1) * P].astype(f32)
                pre += icd[:, None] @ b_all[layer].astype(f32)[None, :]
                scl = (plan.scc[c] if layer < plan.n_layers - 1 else
                       plan.scd[c])[:, b]
                hb = np.maximum(pre * scl[:, None], 0).astype(f16)
                h4_blocks[c][b] = hb
                lo = c * NPC + b * P
                hi = min(lo + P, min((c + 1) * NPC, plan.N))
                if hi > lo:
                    nxt[lo:hi] = hb[:hi - lo]
        htab = nxt
    pgT = np.zeros((plan.D, P), dtype=f32)
    for c in range(plan.NC):
        for b in range(NB):
            hb = h4_blocks[c][b]
            SgT = ((iota.astype(np.float32) == plan.gidf[c][:, b:b + 1]) * plan.invc[c][:, b:b + 1]).astype(f16)
            pgT += hb.astype(f32).T @ SgT.astype(f32)
    out = pgT.T @ Wc.astype(f32) + bc[None, :]
    return out[:plan.G].astype(f32)


# --------------------------------------------------------------------------
# Bass/Tile kernel builder.
# --------------------------------------------------------------------------

def build_inputs(plan: Plan, x, W_all, b_all, Wc, bc):
    """Per-core in_maps for run_bass_kernel_spmd."""
    iota = np.tile(np.arange(P, dtype=np.float16)[None, :], (P, 1))
    common = {
        "x16": np.ascontiguousarray((x * plan.c_src[:, None]).astype(np.float16)),
        "w16": np.ascontiguousarray(W_all.astype(np.float16)),
        "b16": np.ascontiguousarray(b_all.astype(np.float16).reshape(1, -1)),
        "wc32": np.ascontiguousarray(Wc.astype(np.float32)),
        "bc32": np.ascontiguousarray(bc.astype(np.float32).reshape(1, -1)),
        "iota16": iota,
    }
    maps = []
    for c in range(plan.NC):
        m = dict(common)
        m["src16"] = plan.src16[c]
        m["dl16"] = plan.dl[c]
        m["scc32"] = plan.scc[c]
        m["scd32"] = plan.scd[c]
        m["icd16"] = plan.icd[c]
        m["gid16"] = plan.gidf[c]
        m["ivc16"] = plan.invc[c]
        maps.append(m)
    return maps


def build_nc(plan: Plan, num_swdge_queues=4, use_collectives=True):
    import concourse.bass as bass
    import concourse.tile as tile
    from concourse import bacc, mybir
    from concourse.tile_rust import add_dep_helper

    def _inst(i):
        return i.ins if hasattr(i, "ins") and not hasattr(i, "engine") else i

    f16, f32, i16 = mybir.dt.float16, mybir.dt.float32, mybir.dt.int16
    NB, K0, K1, SB = plan.NB, plan.K0, plan.K1, plan.SB
    KT = K0 + K1
    NL = plan.n_layers

    nc = bacc.Bacc(
        "TRN2",
        target_bir_lowering=False,
        debug=False,
        num_devices=plan.NC,
        num_swdge_queues=num_swdge_queues,
        dynamic_dma_scratch_size=49152,
    )
    rg = [list(range(plan.NC))]

    # ---- DRAM I/O ----
    x16 = nc.dram_tensor("x16", [plan.N, P], f16, kind="ExternalInput")
    w16 = nc.dram_tensor("w16", [NL, P, P], f16, kind="ExternalInput")
    b16 = nc.dram_tensor("b16", [1, NL * P], f16, kind="ExternalInput")
    wc32 = nc.dram_tensor("wc32", [P, plan.C], f32, kind="ExternalInput")
    bc32 = nc.dram_tensor("bc32", [1, plan.C], f32, kind="ExternalInput")
    iota16 = nc.dram_tensor("iota16", [P, P], f16, kind="ExternalInput")
    src16 = nc.dram_tensor("src16", [P, plan.slots // 16], i16, kind="ExternalInput")
    dl16 = nc.dram_tensor("dl16", [P, plan.ntiles], f16, kind="ExternalInput")
    scc32 = nc.dram_tensor("scc32", [P, NB], f32, kind="ExternalInput")
    scd32 = nc.dram_tensor("scd32", [P, NB], f32, kind="ExternalInput")
    icd16 = nc.dram_tensor("icd16", [1, NB * P], f16, kind="ExternalInput")
    gid16 = nc.dram_tensor("gid16", [P, NB], f32, kind="ExternalInput")
    ivc16 = nc.dram_tensor("ivc16", [P, NB], f32, kind="ExternalInput")
    out_d = nc.dram_tensor("out", [plan.G, plan.C], f32, kind="ExternalOutput")

    # internal DRAM: per-layer local slice + gathered full table
    hloc = [nc.dram_tensor(f"hloc{l}", [plan.NPC, P], f16) for l in range(NL - 1)]
    hfull = [nc.dram_tensor(f"hfull{l}", [plan.NPC * plan.NC, P], f16,
                            addr_space="Shared") for l in range(NL - 1)]
    pg_in = nc.dram_tensor("pg_in", [P, P], f32)
    pg_out = nc.dram_tensor("pg_out", [P, P], f32, addr_space="Shared")

    with tile.TileContext(nc) as tc, ExitStack() as ctx:
        const = ctx.enter_context(tc.tile_pool(name="const", bufs=1))
        gpool = ctx.enter_context(tc.tile_pool(name="gather", bufs=5))
        spool = ctx.enter_context(tc.tile_pool(name="sel", bufs=8))
        mpool = ctx.enter_context(tc.tile_pool(name="mt", bufs=4))
        hpool = ctx.enter_context(tc.tile_pool(name="hb", bufs=4))
        h4pool = ctx.enter_context(tc.tile_pool(name="h4", bufs=NB))
        psum_m = ctx.enter_context(tc.tile_pool(name="psum_m", bufs=3, space="PSUM"))
        psum_h = ctx.enter_context(tc.tile_pool(name="psum_h", bufs=2, space="PSUM"))
        psum_g = ctx.enter_context(tc.tile_pool(name="psum_g", bufs=1, space="PSUM"))
        opool = ctx.enter_context(tc.tile_pool(name="outp", bufs=1))

        # ---- constants into SBUF ----
        def cload(tag, dram, shape, dt):
            t = const.tile(shape, dt, tag=tag)
            nc.sync.dma_start(out=t[:], in_=dram[:])
            return t

        IOTA = cload("iota", iota16, [P, P], f16)
        SRC = cload("src", src16, [P, plan.slots // 16], i16)
        DL = cload("dl", dl16, [P, plan.ntiles], f16)
        SCC = cload("scc", scc32, [P, NB], f32)
        SCD = cload("scd", scd32, [P, NB], f32)
        ICD = cload("icd", icd16, [1, NB * P], f16)
        GID = cload("gid", gid16, [P, NB], f32)
        IVC = cload("ivc", ivc16, [P, NB], f32)
        WTS = [cload(f"wt{l}", w16[l], [P, P], f16) for l in range(NL)]
        BROW = cload("brow", b16, [1, NL * P], f16)
        WC = cload("wc", wc32, [P, plan.C], f32)
        BC = cload("bc", bc32, [1, plan.C], f32)
        ONE16 = const.tile([1, P], f16, tag="one16")
        nc.vector.memset(ONE16[:], 1.0)
        ONE32 = const.tile([1, P], f32, tag="one32")
        nc.vector.memset(ONE32[:], 1.0)

        is_eq = mybir.AluOpType.is_equal
        mult = mybir.AluOpType.mult

        def sel_tile(col_src, col):
            """Sg^T[n,g] = (iota_g == gid[n]) * invc[n], fp16 (readout only)."""
            st = spool.tile([P, P], f16, tag="sel")
            nc.vector.tensor_scalar(
                out=st[:], in0=IOTA[:],
                scalar1=col_src[0][:, col:col + 1],
                scalar2=col_src[1][:, col:col + 1],
                op0=is_eq, op1=mult)
            return st

        def sel_batch(t0, nt):
            """Binary S^T for nt consecutive tiles: [P, nt, P] fp16."""
            st = spool.tile([P, 9, P], f16, tag="selb")
            nc.vector.tensor_tensor(
                out=st[:, :nt, :],
                in0=IOTA[:].unsqueeze(1).to_broadcast([P, nt, P]),
                in1=DL[:, t0:t0 + nt].unsqueeze(2).to_broadcast([P, nt, P]),
                op=is_eq)
            return st

        prev_cc = None  # collective inst whose output feeds this layer's gathers
        h4_tiles = [None] * NB
        for layer in range(NL):
            table = x16 if layer == 0 else hfull[layer - 1]
            with nc.named_scope(f"conv{layer}"):
                for sb in range(plan.nsb):
                    blocks = list(plan.sb_blocks(sb))
                    nblk = len(blocks)
                    sbt = nblk * KT  # tiles in this superblock
                    g = gpool.tile([P, SB * KT, P], f16, tag="g")
                    t0 = plan.tile_index(blocks[0], 0)
                    # two gathers: half0 tiles then half1 tiles
                    for half, Kh, ntile_h, toff in (
                        (0, K0, nblk * K0, 0),
                        (1, K1, nblk * K1, nblk * K0),
                    ):
                        n_idx = ntile_h * P
                        col0 = (t0 + toff) * P // 16
                        tab = table[0:plan.HALF, :] if half == 0 else \
                            table[plan.HALF:plan.N, :]
                        gi = nc.gpsimd.dma_gather(
                            g[:, toff:toff + ntile_h, :],
                            tab,
                            SRC[:, col0:col0 + n_idx // 16],
                            n_idx, n_idx, P,
                            queue_num=(2 * sb + half) % num_swdge_queues,
                            single_packet=False,
                        )
                        if prev_cc is not None:
                            add_dep_helper(_inst(gi), _inst(prev_cc), reason="gather after AG")

                    for bl, b in enumerate(blocks):
                        pm = psum_m.tile([P, P], f32, tag="pm")
                        st0 = sel_batch(plan.tile_index(b, 0), K0)
                        st1 = sel_batch(plan.tile_index(b, K0), K1)
                        for t in range(KT):
                            st = st0[:, t, :] if t < K0 else st1[:, t - K0, :]
                            gt = (bl * K0 + t) if t < K0 else \
                                (nblk * K0 + bl * K1 + (t - K0))
                            nc.tensor.matmul(
                                out=pm[:], lhsT=g[:, gt, :], rhs=st,
                                start=(t == 0), stop=(t == KT - 1))
                        mt = mpool.tile([P, P], f16, tag="mt")
                        nc.vector.tensor_copy(out=mt[:], in_=pm[:])
                        ph = psum_h.tile([P, P], f32, tag="ph")
                        nc.tensor.matmul(out=ph[:], lhsT=mt[:], rhs=WTS[layer][:],
                                         start=True, stop=False)
                        nc.tensor.matmul(out=ph[:],
                                         lhsT=ICD[0:1, b * P:(b + 1) * P],
                                         rhs=BROW[0:1, layer * P:(layer + 1) * P],
                                         start=False, stop=True)
                        if layer < NL - 1:
                            hb = hpool.tile([P, P], f16, tag="hb")
                        else:
                            hb = h4pool.tile([P, P], f16, tag="h4")
                        scl = SCC if layer < NL - 1 else SCD
                        nc.scalar.activation(
                            out=hb[:], in_=ph[:],
                            func=mybir.ActivationFunctionType.Relu,
                            scale=scl[:, b:b + 1])
                        if layer < NL - 1:
                            rows = min(plan.NPC - b * P, P)
                            nc.sync.dma_start(
                                out=hloc[layer][b * P:b * P + rows, :],
                                in_=hb[:rows, :])
                        else:
                            h4_tiles[b] = hb
            if layer < NL - 1:
                if use_collectives:
                    prev_cc = nc.gpsimd.collective_compute(
                        "AllGather", mybir.AluOpType.bypass,
                        ins=[hloc[layer].ap().opt()], outs=[hfull[layer].ap().opt()],
                        replica_groups=rg)
                else:
                    assert plan.NC == 1
                    prev_cc = nc.sync.dma_start(out=hfull[layer][:, :],
                                                in_=hloc[layer][:, :])

        # ---- readout ----
        with nc.named_scope("readout"):
            pg = psum_g.tile([P, P], f32, tag="pg")
            for b in range(NB):
                sg = sel_tile((GID, IVC), b)
                nc.tensor.matmul(out=pg[:], lhsT=h4_tiles[b][:], rhs=sg[:],
                                 start=(b == 0), stop=(b == NB - 1))
            pgs = opool.tile([P, P], f32, tag="pgs")
            nc.vector.tensor_copy(out=pgs[:], in_=pg[:])
            wr = nc.sync.dma_start(out=pg_in[:, :], in_=pgs[:])
            if use_collectives:
                cc = nc.gpsimd.collective_compute(
                    "AllReduce", mybir.AluOpType.add,
                    ins=[pg_in.ap().opt()], outs=[pg_out.ap().opt()],
                    replica_groups=rg)
            else:
                cc = nc.sync.dma_start(out=pg_out[:, :], in_=pg_in[:, :])
            hgT = opool.tile([P, P], f32, tag="hgT")
            rd = nc.sync.dma_start(out=hgT[:], in_=pg_out[:, :])
            add_dep_helper(_inst(rd), _inst(cc), reason="read after AR")
            po = psum_g.tile([P, plan.C], f32, tag="po")
            nc.tensor.matmul(out=po[:plan.G, :], lhsT=hgT[:, :plan.G], rhs=WC[:],
                             start=True, stop=False)
            nc.tensor.matmul(out=po[:plan.G, :], lhsT=ONE32[0:1, :plan.G], rhs=BC[:],
                             start=False, stop=True)
            ob = opool.tile([P, plan.C], f32, tag="ob")
            nc.vector.tensor_copy(out=ob[:plan.G, :], in_=po[:plan.G, :])
            nc.sync.dma_start(out=out_d[:, :], in_=ob[:plan.G, :])

    nc.compile()
    return nc


# --------------------------------------------------------------------------
# Entry point.
# --------------------------------------------------------------------------

_CACHE = {}


def _get_compiled(plan_key, plan):
    if plan_key not in _CACHE:
        _CACHE[plan_key] = build_nc(plan)
    return _CACHE[plan_key]


def kernel(x, W0, b0, Ws, bs, Wc, bc, edge_index, graph_ids):
    x = np.asarray(x)
    edge_index = np.asarray(edge_index)
    graph_ids = np.asarray(graph_ids)
    W_all = np.concatenate([np.asarray(W0)[None], np.asarray(Ws)], axis=0)
    b_all = np.concatenate([np.asarray(b0)[None], np.asarray(bs)], axis=0)
    Wc, bc = np.asarray(Wc), np.asarray(bc)

    plan = make_plan(x, edge_index, graph_ids)
    key = (plan.N, plan.E, plan.G, plan.K0, plan.K1, plan.n_layers)
    nc = _get_compiled(key, plan)

    from concourse.bass_utils import run_bass_kernel_spmd
    in_maps = build_inputs(plan, x, W_all, b_all, Wc, bc)
    res = run_bass_kernel_spmd(nc, in_maps, core_ids=list(range(plan.NC)))
    return res.results[0]["out"].astype(np.float32)

